# revision 56
# baseline (speedup 1.0000x reference)
"""RNN-T joiner (nn_CombinationModel_53154515256115) as a Bass/Tile SPMD kernel
for 8 Trainium2 NeuronCores.

Algorithm
---------
The reference computes, for each valid (b, t, u):
    out[b,t,u] = relu(enc[b,t] @ Wj1_enc + pred[b,u] @ Wj1_pred + bj1) @ Wj2 + bj2
The joint pre-activation factors into a per-(b,t) term A and a per-(b,u) term
Pp, collapsing the first joiner matmul to ~2 GFLOP. The remaining dominant
work is the [N,640] @ [640,1056] output matmul (bf16 on the PE), the ragged
broadcast-add expansion, and the output write (bf16 on device, upcast + bias
on host).

Sharding (SPMD-uniform)
-----------------------
Core c takes encoder frames t with t % 8 == c from every batch. Every core
then runs an identical program shape: per batch b it owns G[b] = ceil(T_b/8)
frame-groups of (U_b+1) rows each (8134 rows/core; rows of garbage frame-
groups where c + 8g >= T_b are dropped on the host). The tiny prediction
network (263 rows) is computed replicated on every core.

Engine plan (per core)
----------------------
  PE   : warmup matmuls (p-state ramp, bridges the initial DMA wait) ->
         h1 -> pred2 -> Pp -> AT (with batch-0 expansion chained per j) ->
         64 main output tiles, gap-free.
  DVE  : Pp psum evacuation + ragged expansion adds  at[g] + pp[u]
         (+ same-queue ReLU for the tile0-critical first chunk)
  Pool : ReLU of the expanded chunks
  Act  : tanh/identity for the small stages + PSUM->SBUF bf16 evacuation
  DMA  : host-packed bf16 weights in critical-path order, 2-way splits of
         wp1/wp2/wj1p so each stage starts on partial arrivals, bf16 output
"""

from contextlib import ExitStack

import numpy as np
import ml_dtypes

import concourse.bass as bass
import concourse.mybir as mybir
import concourse.tile as tile
from concourse import bacc
from concourse.bass_utils import run_bass_kernel_spmd

F32 = mybir.dt.float32
BF16 = mybir.dt.bfloat16
AF = mybir.ActivationFunctionType
ADD = mybir.AluOpType.add
NPBF = ml_dtypes.bfloat16

# ---------------------------------------------------------------- constants
B, T, U = 8, 300, 40
E, P, J, V = 512, 640, 640, 1056
H, DEMB = 2, 256
ENC_SIZES = [300, 280, 260, 240, 220, 210, 205, 200]
TGT_SIZES = [40, 38, 35, 33, 30, 28, 26, 25]
NCORES = 8
N_FLAT = 64385

G = [(t + NCORES - 1) // NCORES for t in ENC_SIZES]       # groups/core/batch
UB1 = [u + 1 for u in TGT_SIZES]                          # u-extent per batch
RBV = [G[b] * UB1[b] for b in range(B)]                   # valid rows/batch
ROWS = sum(RBV)                                           # 8134 rows/core
GT_TOT = sum(G)                                           # 242 enc frames/core
GT_PAD = 256
UB_TOT = sum(UB1)                                         # 263 pred rows
OFF_T = [0]
for b in range(B):
    OFF_T.append(OFF_T[-1] + G[b])
OFF_U = [0]
for b in range(B):
    OFF_U.append(OFF_U[-1] + UB1[b])
OFF_R = [0]
for b in range(B):
    OFF_R.append(OFF_R[-1] + RBV[b])

KE = E // 128                 # 4 k-tiles for the 512-dim (enc / embedding)
KJ = J // 128                 # 5 k-tiles for the 640-dim (pred / joiner)
V_CHUNKS = [(0, 512), (512, 512), (1024, V - 1024)]
NTILES = (ROWS + 127) // 128

_cache = {}


def _build():
    nc = bacc.Bacc("TRN2", target_bir_lowering=False, debug=False,
                   num_devices=NCORES)

    # host-packed bf16 inputs, [128, k*width] k-tile-major
    et_d = nc.dram_tensor("et", [128, KE * UB_TOT], BF16, kind="ExternalInput").ap()
    wp1_d = nc.dram_tensor("wp1", [128, KE * P], BF16, kind="ExternalInput").ap()
    enct_d = nc.dram_tensor("enct", [128, KE * GT_PAD], BF16, kind="ExternalInput").ap()
    wj1e_d = nc.dram_tensor("wj1e", [128, KE * J], BF16, kind="ExternalInput").ap()
    wp2_d = nc.dram_tensor("wp2", [128, KJ * P], BF16, kind="ExternalInput").ap()
    wj1p_d = nc.dram_tensor("wj1p", [128, KJ * J], BF16, kind="ExternalInput").ap()
    wj2_d = nc.dram_tensor("wj2", [128, KJ * V], BF16, kind="ExternalInput").ap()
    bp1_d = nc.dram_tensor("b_p1", [P], F32, kind="ExternalInput").ap()
    bp2_d = nc.dram_tensor("b_p2", [P], F32, kind="ExternalInput").ap()
    bj1_d = nc.dram_tensor("b_j1", [J], F32, kind="ExternalInput").ap()
    out_d = nc.dram_tensor("out", [ROWS, V], BF16, kind="ExternalOutput").ap()

    with tile.TileContext(nc) as tc, ExitStack() as ctx:
        persist = ctx.enter_context(tc.tile_pool(name="persist", bufs=1))
        stage = ctx.enter_context(tc.tile_pool(name="stage", bufs=1))
        expand = ctx.enter_context(tc.tile_pool(name="expand", bufs=8))
        outp = ctx.enter_context(tc.tile_pool(name="outp", bufs=3))
        ps_small = ctx.enter_context(tc.tile_pool(name="ps_small", bufs=2, space="PSUM"))
        ps_main = ctx.enter_context(tc.tile_pool(name="ps_main", bufs=2, space="PSUM"))

        # ---------------- persistent SBUF state
        wj2_t = persist.tile([128, KJ * V], BF16, tag="wj2", name="wj2")
        at_t = [persist.tile([128, GT_TOT], F32, tag=f"at_{j}", name=f"at_{j}")
                for j in range(KJ)]
        pp_t = [persist.tile([128, UB_TOT], F32, tag=f"pp_{j}", name=f"pp_{j}")
                for j in range(KJ)]
        ht_t = [persist.tile([128, ROWS], BF16, tag=f"ht_{j}", name=f"ht_{j}")
                for j in range(KJ)]

        # ---------------- PE warmup: burn the p-state ramp during DMA waits
        warm = stage.tile([128, 128], BF16, tag="warm", name="warm")
        nc.gpsimd.memset(warm[:], 0.0)
        ps_w = ps_small.tile([128, 512], F32, tag="ps_s", name="ps_warm")
        NWARM = 45
        for i in range(NWARM):
            nc.tensor.matmul(out=ps_w[:, 0:128], lhsT=warm[:], rhs=warm[:],
                             start=(i == 0), stop=(i == NWARM - 1))
        # preload the activation table (exp_and_others: tanh+identity+copy+relu)
        # during the DMA dead time instead of stalling the first h1 tanh
        warm_act = stage.tile([128, 1], BF16, tag="warm_act", name="warm_act")
        nc.scalar.activation(warm_act[:], warm[:, 0:1], AF.Tanh, scale=1.0)

        # ---------------- input DMAs in critical-path order
        et_t = stage.tile([128, KE * UB_TOT], BF16, tag="et", name="et")
        wp1_t = stage.tile([128, KE * P], BF16, tag="wp1", name="wp1")
        enct_t = stage.tile([128, KE * GT_PAD], BF16, tag="enct", name="enct")
        wj1e_t = stage.tile([128, KE * J], BF16, tag="wj1e", name="wj1e")
        wp2_t = stage.tile([128, KJ * P], BF16, tag="wp2", name="wp2")
        wj1p_t = stage.tile([128, KJ * J], BF16, tag="wj1p", name="wj1p")
        bp1_t = stage.tile([128, 5], F32, tag="bp1")
        bp2_t = stage.tile([128, 5], F32, tag="bp2")
        bj1_t = stage.tile([128, 5], F32, tag="bj1")

        # critical-path order: h1 deps, pred2 deps, AT deps, pp deps, then wj2
        # split per k-tile so tile0's first matmuls don't wait the full 1.35MB
        # et via the idle Act queue: its DGE chain starts ~0.6us before SP's,
        # pulling the whole prediction-network chain forward
        nc.scalar.dma_start(out=et_t[:], in_=et_d)
        nc.sync.dma_start(out=wp1_t[:, 0:2 * P], in_=wp1_d[:, 0:2 * P])
        nc.sync.dma_start(out=wp1_t[:, 2 * P:], in_=wp1_d[:, 2 * P:])
        nc.sync.dma_start(out=bp1_t[:], in_=bp1_d.rearrange("(a p) -> p a", p=128))
        nc.sync.dma_start(out=wp2_t[:, 0:3 * P], in_=wp2_d[:, 0:3 * P])
        nc.sync.dma_start(out=wp2_t[:, 3 * P:], in_=wp2_d[:, 3 * P:])
        nc.sync.dma_start(out=bp2_t[:], in_=bp2_d.rearrange("(a p) -> p a", p=128))
        nc.sync.dma_start(out=wj1p_t[:, 0:3 * J], in_=wj1p_d[:, 0:3 * J])
        nc.sync.dma_start(out=wj1p_t[:, 3 * J:], in_=wj1p_d[:, 3 * J:])
        nc.sync.dma_start(out=enct_t[:], in_=enct_d)
        nc.sync.dma_start(out=wj1e_t[:], in_=wj1e_d)
        nc.sync.dma_start(out=bj1_t[:], in_=bj1_d.rearrange("(a p) -> p a", p=128))
        for k in range(KJ):
            nc.sync.dma_start(out=wj2_t[:, k * V:(k + 1) * V],
                              in_=wj2_d[:, k * V:(k + 1) * V])

        # pred psums alternate between both PSUM pools (both idle pre-main)
        def pred_ps(j, width):
            pool = ps_small if j % 2 == 0 else ps_main
            tag = "ps_s" if j % 2 == 0 else "ps_out"
            return pool.tile([128, width], F32, tag=tag, name=f"ps_pred{j}")

        # ---------------- prediction network + AT (PE + Act), 263/242 rows
        h1_t = [stage.tile([128, UB_TOT], BF16, tag=f"h1_{j}", name=f"h1_{j}")
                for j in range(KJ)]
        for j in range(KJ):
            ps = pred_ps(j, UB_TOT)
            for k in range(KE):
                nc.tensor.matmul(out=ps[:],
                                 lhsT=wp1_t[:, k * P + j * 128: k * P + (j + 1) * 128],
                                 rhs=et_t[:, k * UB_TOT:(k + 1) * UB_TOT],
                                 start=(k == 0), stop=(k == KE - 1))
            nc.scalar.activation(h1_t[j][:], ps[:], AF.Tanh,
                                 bias=bp1_t[:, j:j + 1], scale=1.0)

        pred_t = [stage.tile([128, UB_TOT], BF16, tag=f"pred_{j}", name=f"pred_{j}")
                  for j in range(KJ)]
        for j in range(KJ):
            ps = pred_ps(j, UB_TOT)
            for k in range(KJ):
                nc.tensor.matmul(out=ps[:],
                                 lhsT=wp2_t[:, k * P + j * 128: k * P + (j + 1) * 128],
                                 rhs=h1_t[k][:],
                                 start=(k == 0), stop=(k == KJ - 1))
            nc.scalar.activation(pred_t[j][:], ps[:], AF.Tanh,
                                 bias=bp2_t[:, j:j + 1], scale=1.0)

        # ---------------- ragged expansion (DVE add + Pool/Act relu), chunked
        def emit_chunk(b, j, g0, g1, relu_act=False, relu_dve=False):
            u1 = UB1[b]
            gn = g1 - g0
            rn = gn * u1
            r0 = OFF_R[b] + g0 * u1
            tmp = expand.tile([128, 16 * 41], BF16, tag="tmp", name="tmp")
            nc.vector.tensor_tensor(
                out=tmp[:, 0:rn].rearrange("p (g u) -> p g u", g=gn),
                in0=at_t[j][:, OFF_T[b] + g0: OFF_T[b] + g1][:, :, None]
                    .to_broadcast([128, gn, u1]),
                in1=pp_t[j][:, OFF_U[b]: OFF_U[b] + u1][:, None, :]
                    .to_broadcast([128, gn, u1]),
                op=ADD)
            if relu_act:
                nc.scalar.activation(ht_t[j][:, r0:r0 + rn], tmp[:, 0:rn],
                                     AF.Relu, scale=1.0)
            elif relu_dve:
                nc.vector.tensor_scalar_max(ht_t[j][:, r0:r0 + rn], tmp[:, 0:rn], 0.0)
            else:
                nc.gpsimd.tensor_scalar_max(ht_t[j][:, r0:r0 + rn], tmp[:, 0:rn], 0.0)

        # Pp; evac + first expansion chunk interleaved per j so tile0's
        # rows are ready the moment the PE finishes the pp stage
        B0C = 4
        for j in range(KJ):
            ps = pred_ps(j, UB_TOT)
            for k in range(KJ):
                nc.tensor.matmul(out=ps[:],
                                 lhsT=wj1p_t[:, k * J + j * 128: k * J + (j + 1) * 128],
                                 rhs=pred_t[k][:],
                                 start=(k == 0), stop=(k == KJ - 1))
            nc.vector.tensor_copy(pp_t[j][:], ps[:])

        # A = enc @ Wj1_enc + bj1 (after pp: its inputs arrive last), with the
        # batch-0 expansion chunk chained per j so tile0 is ready immediately
        for j in range(KJ):
            ps = pred_ps(j, GT_TOT)
            for k in range(KE):
                nc.tensor.matmul(out=ps[:],
                                 lhsT=wj1e_t[:, k * J + j * 128: k * J + (j + 1) * 128],
                                 rhs=enct_t[:, k * GT_PAD: k * GT_PAD + GT_TOT],
                                 start=(k == 0), stop=(k == KE - 1))
            nc.scalar.activation(at_t[j][:], ps[:], AF.Identity,
                                 bias=bj1_t[:, j:j + 1], scale=1.0)
            emit_chunk(0, j, 0, B0C, relu_dve=True)
            emit_chunk(0, j, B0C, B0C + 6, relu_act=False)

        # chunk schedule: small chunks early (low first-tile latency)
        chunk_q = []
        for b in range(B):
            g0 = B0C + 6 if b == 0 else 0
            steps = [6, 6] if b == 0 else [12]
            si = 0
            while g0 < G[b]:
                step = steps[si] if si < len(steps) else 12
                si += 1
                g1 = min(g0 + step, G[b])
                chunk_q.append((b, g0, g1))
                g0 = g1
        cover_q = [OFF_R[b] + g1 * UB1[b] for (b, g0, g1) in chunk_q]

        def emit_group(i):
            b, g0, g1 = chunk_q[i]
            for j in range(KJ):
                emit_chunk(b, j, g0, g1, relu_act=False)

        # ---------------- main loop: 64 output tiles, PE gap-free
        nextc = 0

        def emit_main_tile(rt):
            m = min(128, ROWS - rt * 128)
            ps = ps_main.tile([128, V], F32, tag="ps_out", name="ps_out")
            for k in range(KJ):
                for (c0, cn) in V_CHUNKS:
                    nc.tensor.matmul(
                        out=ps[0:m, c0:c0 + cn],
                        lhsT=ht_t[k][:, rt * 128: rt * 128 + m],
                        rhs=wj2_t[:, k * V + c0: k * V + c0 + cn],
                        start=(k == 0), stop=(k == KJ - 1))
            osb = outp.tile([128, V], BF16, tag="osb", name="osb")
            nc.scalar.activation(osb[0:m], ps[0:m], AF.Copy, scale=1.0)
            nc.sync.dma_start(out=out_d[rt * 128: rt * 128 + m, :], in_=osb[0:m])

        def emit_last_tile(rt):
            # chunk-major: evac+DMA each V-chunk while the next chunk's
            # matmuls still run; the drain chain then hangs off the tiny
            # 32-column chunk instead of the whole tile
            m = ROWS - rt * 128
            osb = outp.tile([128, V], BF16, tag="osb", name="osb")
            pss = []
            for ci, (c0, cn) in enumerate(V_CHUNKS):
                # independent PSUM tiles per chunk: no WAR hazard between a
                # chunk's evacuation and the next chunk's matmuls
                pool = ps_main if ci == 0 else ps_small
                ps = pool.tile([128, cn], F32,
                               tag=("ps_out" if ci == 0 else "ps_s"),
                               name=f"ps_last{ci}")
                pss.append(ps)
                for k in range(KJ):
                    nc.tensor.matmul(
                        out=ps[0:m, :],
                        lhsT=ht_t[k][:, rt * 128: rt * 128 + m],
                        rhs=wj2_t[:, k * V + c0: k * V + c0 + cn],
                        start=(k == 0), stop=(k == KJ - 1))
                if ci == 0:
                    nc.scalar.activation(osb[0:m, 0:512], ps[0:m, :],
                                         AF.Copy, scale=1.0)
                    nc.sync.dma_start(out=out_d[rt * 128: rt * 128 + m, 0:512],
                                      in_=osb[0:m, 0:512])
            # cols 512:1056 evacuate on DVE+Act in parallel, single DMA
            nc.vector.tensor_copy(osb[0:m, 512:1024], pss[1][0:m, :])
            nc.scalar.activation(osb[0:m, 1024:V], pss[2][0:m, :],
                                 AF.Copy, scale=1.0)
            nc.sync.dma_start(out=out_d[rt * 128: rt * 128 + m, 512:V],
                              in_=osb[0:m, 512:V])

        for rt in range(NTILES):
            # hard deadline + lookahead: cover rows of tile rt+4 before rt runs
            want = min((rt + 4) * 128, ROWS)
            while nextc < len(chunk_q) and (cover_q[nextc - 1] if nextc else (B0C + 6) * 41) < want:
                emit_group(nextc)
                nextc += 1
            if rt == NTILES - 1:
                emit_last_tile(rt)
            else:
                emit_main_tile(rt)

    nc.compile()
    return nc


def _host_inputs(inputs):
    """Build per-core in_maps: bf16-packed, k-tile-major transposed weights."""
    enc = np.ascontiguousarray(np.asarray(inputs["encoder_states"], dtype=np.float32))
    targets = np.asarray(inputs["targets"]).astype(np.int64)
    emb = np.asarray(inputs["emb"], dtype=np.float32)

    def packT(w, ktiles, width):
        # w [k*128, width] fp32 -> [128, k*width] bf16 with
        # out[p, k*width + o] = w[k*128 + p, o]
        out = np.empty((128, ktiles * width), dtype=NPBF)
        for k in range(ktiles):
            out[:, k * width:(k + 1) * width] = w[k * 128:(k + 1) * 128, :].astype(NPBF)
        return out

    wj1 = np.asarray(inputs["W_j1"], dtype=np.float32)

    # embedding context e[(b,u), 512] = [emb[y_{u-1}], emb[y_{u-2}]] masked
    ext = np.zeros((B, U + H), np.int64)
    ext[:, H:] = targets
    e = np.zeros((UB_TOT, H * DEMB), np.float32)
    for b in range(B):
        for u in range(UB1[b]):
            c0, c1 = ext[b, u + 1], ext[b, u]
            if c0 != 0:
                e[OFF_U[b] + u, 0:DEMB] = emb[c0]
            if c1 != 0:
                e[OFF_U[b] + u, DEMB:] = emb[c1]

    common = {
        "et": packT(e.T.copy(), KE, UB_TOT).copy(),
        "wp1": packT(np.asarray(inputs["W_pred1"], np.float32), KE, P),
        "wp2": packT(np.asarray(inputs["W_pred2"], np.float32), KJ, P),
        "wj1e": packT(wj1[0:E], KE, J),
        "wj1p": packT(wj1[E:], KJ, J),
        "wj2": packT(np.asarray(inputs["W_j2"], np.float32), KJ, V),
        "b_p1": np.asarray(inputs["b_pred1"], dtype=np.float32),
        "b_p2": np.asarray(inputs["b_pred2"], dtype=np.float32),
        "b_j1": np.asarray(inputs["b_j1"], dtype=np.float32),
    }
    in_maps = []
    for c in range(NCORES):
        enc_sel = np.zeros((GT_PAD, E), np.float32)
        for b in range(B):
            ts = c + NCORES * np.arange(G[b])
            valid = ts < ENC_SIZES[b]
            rows = np.where(valid)[0]
            enc_sel[OFF_T[b] + rows] = enc[b, ts[valid]]
        in_maps.append({"enct": packT(enc_sel.T.copy(), KE, GT_PAD), **common})
    return in_maps


def _gather_output(core_outs, inputs):
    fb = np.asarray(inputs["flat_b"]).astype(np.int64)
    ft = np.asarray(inputs["flat_t"]).astype(np.int64)
    fu = np.asarray(inputs["flat_u"]).astype(np.int64)
    bj2 = np.asarray(inputs["b_j2"], dtype=np.float32)
    ub1 = np.asarray(UB1, np.int64)
    off_r = np.asarray(OFF_R[:B], np.int64)
    core = ft % NCORES
    local = off_r[fb] + (ft // NCORES) * ub1[fb] + fu
    out = np.empty((fb.shape[0], V), np.float32)
    for c in range(NCORES):
        m = core == c
        out[m] = core_outs[c][local[m]].astype(np.float32)
    out += bj2[None, :]
    return out


def kernel(**inputs) -> np.ndarray:
    if "nc" not in _cache:
        _cache["nc"] = _build()
    nc = _cache["nc"]
    in_maps = _host_inputs(inputs)
    res = run_bass_kernel_spmd(nc, in_maps, list(range(NCORES))).results
    core_outs = [res[c]["out"] for c in range(NCORES)]
    return _gather_output(core_outs, inputs)
